# revision 7
# baseline (speedup 1.0000x reference)
"""CrossModalFeatureInteraction kernel for Trainium2 (Bass/Tile), 8 NeuronCores.

Computation (per pixel, per batch):
    combined = concat([vis, ir], channel)              # [512]
    x        = relu(W1 @ combined + b1)                # [32]
    residual = W2 @ x + b2                             # [256]
    out      = vis + ir + residual                     # [256]

Sharding: data-parallel over batch. B=16 -> 2 images per core on 8 cores.
Weights are tiny and replicated. Each core streams its 2 images through
SBUF in pixel supertiles; 1x1 convs are matmuls with channels as the
contraction dim and pixels as the moving free dim.

Engine budget tricks (target regime is memory; DMA ~77us/core is the
roofline, so every other engine must stay well under it):
  - Matmuls run in float32r: full-rate (1 col/cycle) PE mode on fp32 bits.
  - b1 rides as the activation bias; an all-zero 33rd W1 column plus
    bias=1.0 makes x's 33rd row == 1.0, so b2 rides as the 33rd row of
    W2 (K=33 second matmul). No separate bias pass.
  - The ir half of the bypass path is added by the PE: an identity
    matmul accumulates ir into the residual's PSUM bank. The only DVE
    work left is one add per output tile (vis + psum) -- fp32
    tensor_tensor is stuck at 1x mode, so halving DVE work matters.
"""

import numpy as np

import concourse.bass as bass
import concourse.mybir as mybir
from concourse import bacc, bass_utils
from concourse.tile import TileContext

# Problem shape (hardcoded per contract)
B, C, H, W = 16, 256, 64, 64
HID = 32
HWPIX = H * W          # 4096 pixels per image
N_CORES = 8
B_PER_CORE = B // N_CORES  # 2

NBIG = 2048            # pixels per DMA supertile (1 MiB per [128, NBIG] f32 tile)
NT = 512               # matmul moving free dim (one PSUM bank of fp32)
KO = 4                 # 512 combined channels / 128 partitions
CCH = 2                # 256 output channels / 128 partitions
HID1 = HID + 1         # hidden + ones row (carries b2 through matmul 2)

F32 = mybir.dt.float32
F32R = mybir.dt.float32r

_cache = {}


def _build(mm_dt: str) -> bass.Bass:
    d = F32R if mm_dt == "f32r" else F32

    nc = bacc.Bacc(
        "TRN2", target_bir_lowering=False, debug=False, num_devices=N_CORES
    )
    vis = nc.dram_tensor("vis", [B_PER_CORE, C, HWPIX], d, kind="ExternalInput")
    ir = nc.dram_tensor("ir", [B_PER_CORE, C, HWPIX], d, kind="ExternalInput")
    w1t = nc.dram_tensor("w1t", [KO, 128, HID1], d, kind="ExternalInput")
    b1 = nc.dram_tensor("b1", [HID1, 1], F32, kind="ExternalInput")
    w2t = nc.dram_tensor("w2t", [HID1, C], d, kind="ExternalInput")
    iden = nc.dram_tensor("iden", [128, 128], d, kind="ExternalInput")
    out = nc.dram_tensor("out", [B_PER_CORE, C, HWPIX], F32, kind="ExternalOutput")

    with TileContext(nc) as tc:
        with (
            tc.tile_pool(name="consts", bufs=1) as cpool,
            tc.tile_pool(name="inbuf", bufs=3) as inpool,
            tc.tile_pool(name="work", bufs=3) as wpool,
            tc.tile_pool(name="outbuf", bufs=2) as opool,
            tc.tile_pool(name="ps1", bufs=2, space="PSUM") as ps1pool,
            tc.tile_pool(name="ps2", bufs=3, space="PSUM") as ps2pool,
        ):
            w1t_sb = cpool.tile([128, KO, HID1], d, tag="w1t")
            nc.sync.dma_start(w1t_sb, w1t[:, :, :].rearrange("ko p m -> p ko m"))
            w2t_sb = cpool.tile([HID1, C], d, tag="w2t")
            nc.sync.dma_start(w2t_sb, w2t[:, :])
            b1_sb = cpool.tile([HID1, 1], F32, tag="b1")
            nc.sync.dma_start(b1_sb, b1[:, :])
            iden_sb = cpool.tile([128, 128], d, tag="iden")
            nc.sync.dma_start(iden_sb, iden[:, :])

            for b in range(B_PER_CORE):
                for j in range(HWPIX // NBIG):
                    jsl = slice(j * NBIG, (j + 1) * NBIG)
                    ins = {}
                    for nm, dram in (("v", vis), ("i", ir)):
                        for c in range(CCH):
                            t = inpool.tile(
                                [128, NBIG], d, tag=f"in_{nm}{c}", name=f"in_{nm}{c}"
                            )
                            nc.sync.dma_start(t, dram[b, c * 128 : (c + 1) * 128, jsl])
                            ins[(nm, c)] = t
                    outs = [
                        opool.tile([128, NBIG], F32, tag=f"out{c}", name=f"outt{c}")
                        for c in range(CCH)
                    ]
                    for js in range(NBIG // NT):
                        sl = slice(js * NT, (js + 1) * NT)
                        ps2s = []
                        # ir bypass rides the PE: psum2[c] starts as I.T @ ir_c
                        for c in range(CCH):
                            ps2 = ps2pool.tile(
                                [128, NT], F32, tag=f"ps2_{c}", name=f"ps2_{c}"
                            )
                            nc.tensor.matmul(
                                ps2,
                                lhsT=iden_sb,
                                rhs=ins[("i", c)][:, sl],
                                start=True,
                                stop=False,
                            )
                            ps2s.append(ps2)
                        ps1 = ps1pool.tile([HID1, NT], F32, tag="ps1", name="ps1")
                        rhs_order = [("v", 0), ("v", 1), ("i", 0), ("i", 1)]
                        for ko, key in enumerate(rhs_order):
                            nc.tensor.matmul(
                                ps1,
                                lhsT=w1t_sb[:, ko],
                                rhs=ins[key][:, sl],
                                start=(ko == 0),
                                stop=(ko == KO - 1),
                            )
                        # x rows 0..31 = relu(W1@c + b1); row 32 = relu(0+1) = 1
                        x_t = wpool.tile([HID1, NT], d, tag="x", name="x_t")
                        nc.scalar.activation(
                            x_t, ps1, mybir.ActivationFunctionType.Relu,
                            bias=b1_sb[:, 0:1],
                        )
                        for c in range(CCH):
                            # psum2[c] += [W2; b2].T @ [x; 1] = residual + b2
                            nc.tensor.matmul(
                                ps2s[c],
                                lhsT=w2t_sb[:, c * 128 : (c + 1) * 128],
                                rhs=x_t,
                                start=False,
                                stop=True,
                            )
                            nc.vector.tensor_add(
                                outs[c][:, sl],
                                ps2s[c],
                                ins[("v", c)][:, sl].bitcast(F32),
                            )
                    for c in range(CCH):
                        nc.sync.dma_start(
                            out[b, c * 128 : (c + 1) * 128, jsl], outs[c]
                        )
    nc.compile()
    return nc


def _get_nc(mm_dt: str) -> bass.Bass:
    key = ("nc", mm_dt)
    if key not in _cache:
        _cache[key] = _build(mm_dt)
    return _cache[key]


def kernel(
    visible_features: np.ndarray,
    infrared_features: np.ndarray,
    W1: np.ndarray,
    b1: np.ndarray,
    W2: np.ndarray,
    b2: np.ndarray,
    _mm_dt: str = "f32r",
    _trace: bool = False,
) -> np.ndarray:
    nc = _get_nc(_mm_dt)

    vis = np.ascontiguousarray(visible_features, dtype=np.float32).reshape(B, C, HWPIX)
    ir = np.ascontiguousarray(infrared_features, dtype=np.float32).reshape(B, C, HWPIX)

    w1t = np.zeros((2 * C, HID1), dtype=np.float32)
    w1t[:, :HID] = W1.astype(np.float32).T
    w1t = np.ascontiguousarray(w1t.reshape(KO, 128, HID1))
    b1r = np.ones((HID1, 1), dtype=np.float32)
    b1r[:HID, 0] = b1.astype(np.float32)
    w2t = np.zeros((HID1, C), dtype=np.float32)
    w2t[:HID] = W2.astype(np.float32).T
    w2t[HID] = b2.astype(np.float32)
    iden = np.eye(128, dtype=np.float32)

    in_maps = []
    for core in range(N_CORES):
        bsl = slice(core * B_PER_CORE, (core + 1) * B_PER_CORE)
        in_maps.append(
            {
                "vis": vis[bsl],
                "ir": ir[bsl],
                "w1t": w1t,
                "b1": b1r,
                "w2t": w2t,
                "iden": iden,
            }
        )

    res = bass_utils.run_bass_kernel_spmd(
        nc, in_maps, core_ids=list(range(N_CORES)), trace=_trace
    )
    if _trace:
        kernel.last_results = res
    outs = [r["out"] for r in res.results]
    return np.concatenate(outs, axis=0).reshape(B, C, H, W)


# revision 8
# speedup vs baseline: 1.1202x; 1.1202x over previous
"""CrossModalFeatureInteraction kernel for Trainium2 (Bass/Tile), 8 NeuronCores.

Computation (per pixel, per batch):
    combined = concat([vis, ir], channel)              # [512]
    x        = relu(W1 @ combined + b1)                # [32]
    residual = W2 @ x + b2                             # [256]
    out      = vis + ir + residual                     # [256]

Sharding: data-parallel over batch. B=16 -> 2 images per core on 8 cores.
Weights are tiny and replicated. Each core streams its 2 images through
SBUF in pixel supertiles; 1x1 convs are matmuls with channels as the
contraction dim and pixels as the moving free dim.

Engine budget tricks (target regime is memory; DMA ~77us/core is the
roofline, so every other engine must stay well under it):
  - Matmuls run in float32r: full-rate (1 col/cycle) PE mode on fp32 bits.
  - b1 rides as the activation bias; an all-zero 33rd W1 column plus
    bias=1.0 makes x's 33rd row == 1.0, so b2 rides as the 33rd row of
    W2 (K=33 second matmul). No separate bias pass.
  - Each supertile runs in two phases: all first-layer matmuls + relus
    into one batched x tile first, then all second-layer matmuls. The
    PE never waits on the ACT relu this way (its consumer runs ~4 tile
    slots behind the producer), so it stays dense and HAM-warm.
  - variant "hybrid": the ir half of the bypass is added by the PE (an
    identity matmul accumulating into the residual PSUM bank), leaving
    DVE one add per output tile. variant "dve": both bypass adds on DVE
    (fp32 tensor_tensor is 1x mode, so this doubles DVE time but frees
    the PE).
"""

import numpy as np

import concourse.bass as bass
import concourse.mybir as mybir
from concourse import bacc, bass_utils
from concourse.tile import TileContext

# Problem shape (hardcoded per contract)
B, C, H, W = 16, 256, 64, 64
HID = 32
HWPIX = H * W          # 4096 pixels per image
N_CORES = 8
B_PER_CORE = B // N_CORES  # 2

NBIG = 2048            # pixels per DMA supertile (1 MiB per [128, NBIG] f32 tile)
NT = 512               # matmul moving free dim (one PSUM bank of fp32)
KO = 4                 # 512 combined channels / 128 partitions
CCH = 2                # 256 output channels / 128 partitions
HID1 = HID + 1         # hidden + ones row (carries b2 through matmul 2)

F32 = mybir.dt.float32
F32R = mybir.dt.float32r

_cache = {}


def _build(mm_dt: str, variant: str) -> bass.Bass:
    d = F32R if mm_dt == "f32r" else F32
    hybrid = variant == "hybrid"

    nc = bacc.Bacc(
        "TRN2", target_bir_lowering=False, debug=False, num_devices=N_CORES
    )
    vis = nc.dram_tensor("vis", [B_PER_CORE, C, HWPIX], d, kind="ExternalInput")
    ir = nc.dram_tensor("ir", [B_PER_CORE, C, HWPIX], d, kind="ExternalInput")
    w1t = nc.dram_tensor("w1t", [KO, 128, HID1], d, kind="ExternalInput")
    b1 = nc.dram_tensor("b1", [HID1, 1], F32, kind="ExternalInput")
    w2t = nc.dram_tensor("w2t", [HID1, C], d, kind="ExternalInput")
    iden = nc.dram_tensor("iden", [128, 128], d, kind="ExternalInput")
    out = nc.dram_tensor("out", [B_PER_CORE, C, HWPIX], F32, kind="ExternalOutput")

    with TileContext(nc) as tc:
        with (
            tc.tile_pool(name="consts", bufs=1) as cpool,
            tc.tile_pool(name="inbuf", bufs=3) as inpool,
            tc.tile_pool(name="work", bufs=2) as wpool,
            tc.tile_pool(name="outbuf", bufs=2) as opool,
            tc.tile_pool(name="ps1", bufs=3, space="PSUM") as ps1pool,
            tc.tile_pool(name="ps2", bufs=2, space="PSUM") as ps2pool,
        ):
            w1t_sb = cpool.tile([128, KO, HID1], d, tag="w1t")
            nc.sync.dma_start(w1t_sb, w1t[:, :, :].rearrange("ko p m -> p ko m"))
            w2t_sb = cpool.tile([HID1, C], d, tag="w2t")
            nc.sync.dma_start(w2t_sb, w2t[:, :])
            b1_sb = cpool.tile([HID1, 1], F32, tag="b1")
            nc.sync.dma_start(b1_sb, b1[:, :])
            iden_sb = cpool.tile([128, 128], d, tag="iden")
            nc.sync.dma_start(iden_sb, iden[:, :])

            for b in range(B_PER_CORE):
                for j in range(HWPIX // NBIG):
                    jsl = slice(j * NBIG, (j + 1) * NBIG)
                    ins = {}
                    for nm, dram in (("v", vis), ("i", ir)):
                        for c in range(CCH):
                            t = inpool.tile(
                                [128, NBIG], d, tag=f"in_{nm}{c}", name=f"in_{nm}{c}"
                            )
                            nc.sync.dma_start(t, dram[b, c * 128 : (c + 1) * 128, jsl])
                            ins[(nm, c)] = t
                    outs = [
                        opool.tile([128, NBIG], F32, tag=f"out{c}", name=f"outt{c}")
                        for c in range(CCH)
                    ]
                    # Phase A: first layer for the whole supertile.
                    # x rows 0..31 = relu(W1@c + b1); row 32 = relu(0+1) = 1
                    x_big = wpool.tile([HID1, NBIG], d, tag="x", name="x_big")
                    rhs_order = [("v", 0), ("v", 1), ("i", 0), ("i", 1)]
                    for js in range(NBIG // NT):
                        sl = slice(js * NT, (js + 1) * NT)
                        ps1 = ps1pool.tile([HID1, NT], F32, tag="ps1", name="ps1")
                        for ko, key in enumerate(rhs_order):
                            nc.tensor.matmul(
                                ps1,
                                lhsT=w1t_sb[:, ko],
                                rhs=ins[key][:, sl],
                                start=(ko == 0),
                                stop=(ko == KO - 1),
                            )
                        nc.scalar.activation(
                            x_big[:, sl], ps1, mybir.ActivationFunctionType.Relu,
                            bias=b1_sb[:, 0:1],
                        )
                    # Phase B: second layer + bypass adds.
                    for js in range(NBIG // NT):
                        sl = slice(js * NT, (js + 1) * NT)
                        for c in range(CCH):
                            ps2 = ps2pool.tile(
                                [128, NT], F32, tag=f"ps2_{c}", name=f"ps2_{c}"
                            )
                            if hybrid:
                                # psum2 = I.T @ ir_c  (ir bypass on the PE)
                                nc.tensor.matmul(
                                    ps2,
                                    lhsT=iden_sb,
                                    rhs=ins[("i", c)][:, sl],
                                    start=True,
                                    stop=False,
                                )
                            # psum2 += [W2; b2].T @ [x; 1] = residual + b2
                            nc.tensor.matmul(
                                ps2,
                                lhsT=w2t_sb[:, c * 128 : (c + 1) * 128],
                                rhs=x_big[:, sl],
                                start=not hybrid,
                                stop=True,
                            )
                            if hybrid:
                                nc.vector.tensor_add(
                                    outs[c][:, sl],
                                    ps2,
                                    ins[("v", c)][:, sl].bitcast(F32),
                                )
                            else:
                                s_t = wpool.tile([128, NT], F32, tag="s", name="s_t")
                                nc.vector.tensor_add(
                                    s_t,
                                    ins[("v", c)][:, sl].bitcast(F32),
                                    ins[("i", c)][:, sl].bitcast(F32),
                                )
                                nc.vector.tensor_add(outs[c][:, sl], ps2, s_t)
                    for c in range(CCH):
                        nc.sync.dma_start(
                            out[b, c * 128 : (c + 1) * 128, jsl], outs[c]
                        )
    nc.compile()
    return nc


def _get_nc(mm_dt: str, variant: str) -> bass.Bass:
    key = ("nc", mm_dt, variant)
    if key not in _cache:
        _cache[key] = _build(mm_dt, variant)
    return _cache[key]


def kernel(
    visible_features: np.ndarray,
    infrared_features: np.ndarray,
    W1: np.ndarray,
    b1: np.ndarray,
    W2: np.ndarray,
    b2: np.ndarray,
    _mm_dt: str = "f32r",
    _variant: str = "hybrid",
    _trace: bool = False,
) -> np.ndarray:
    nc = _get_nc(_mm_dt, _variant)

    vis = np.ascontiguousarray(visible_features, dtype=np.float32).reshape(B, C, HWPIX)
    ir = np.ascontiguousarray(infrared_features, dtype=np.float32).reshape(B, C, HWPIX)

    w1t = np.zeros((2 * C, HID1), dtype=np.float32)
    w1t[:, :HID] = W1.astype(np.float32).T
    w1t = np.ascontiguousarray(w1t.reshape(KO, 128, HID1))
    b1r = np.ones((HID1, 1), dtype=np.float32)
    b1r[:HID, 0] = b1.astype(np.float32)
    w2t = np.zeros((HID1, C), dtype=np.float32)
    w2t[:HID] = W2.astype(np.float32).T
    w2t[HID] = b2.astype(np.float32)
    iden = np.eye(128, dtype=np.float32)

    in_maps = []
    for core in range(N_CORES):
        bsl = slice(core * B_PER_CORE, (core + 1) * B_PER_CORE)
        in_maps.append(
            {
                "vis": vis[bsl],
                "ir": ir[bsl],
                "w1t": w1t,
                "b1": b1r,
                "w2t": w2t,
                "iden": iden,
            }
        )

    res = bass_utils.run_bass_kernel_spmd(
        nc, in_maps, core_ids=list(range(N_CORES)), trace=_trace
    )
    if _trace:
        kernel.last_results = res
    outs = [r["out"] for r in res.results]
    return np.concatenate(outs, axis=0).reshape(B, C, H, W)
